# revision 20
# baseline (speedup 1.0000x reference)
"""Causal multi-head self-attention (RoPE) Trainium2 Bass kernel.

Sharding: 8 cores = 2 batches x 4 head-groups (4 heads each).
Per core: QKV projections for its head slice, RoPE, causal flash-style
attention (S^T orientation, ones-row softmax denominator), O-projection
partial, then a per-qtile ReduceScatter (bf16) over the 4 cores of each
batch.

Fast path vs v1: bf16 operands everywhere (fp32 PSUM accumulation),
host-side x transpose + cos/sin tables, phase A/B interleaving to keep
the PE warm (HAM), row-tiled K=64 S^T matmuls (two heads concurrent in
PE quadrants), exp on [128,1024] psum tiles, multiplicative causal mask
on GPSIMD, reciprocal_approx_fast + K=1 broadcast matmul for softmax
denominators.
"""

import sys, math

sys.path.insert(0, '/opt/trn_rl_repo')
import numpy as np

B, S, D, H, DK = 2, 2048, 1024, 16, 64
HC = 4            # heads per core
E = HC * DK       # 256 local projection width
NSC = S // 128    # 16 s-chunks
NQT = S // 256    # 8 q-tiles of 256
ROPE_THETA = 10000.0

_compiled = None
import os
DEBUG = bool(int(os.environ.get("KDEBUG", "0")))


def _build():
    import concourse.bass as bass
    import concourse.tile as tile
    from concourse import bacc, mybir
    from concourse.masks import make_identity

    F32 = mybir.dt.float32
    F32R = mybir.dt.float32r
    BF = mybir.dt.bfloat16
    AF = mybir.ActivationFunctionType
    ALU = mybir.AluOpType

    nc = bacc.Bacc()

    xt_d = nc.dram_tensor("xt", [D, S], BF, kind="ExternalInput")
    wqk_d = nc.dram_tensor("wqk", [D, 2 * E], BF, kind="ExternalInput")
    wv_d = nc.dram_tensor("wv", [D, E], BF, kind="ExternalInput")
    wo_d = nc.dram_tensor("wo", [E, D], BF, kind="ExternalInput")
    cos_d = nc.dram_tensor("cost", [128, NSC * 32], BF, kind="ExternalInput")
    sin_d = nc.dram_tensor("sint", [128, NSC * 32], BF, kind="ExternalInput")
    nsin_d = nc.dram_tensor("nsint", [128, NSC * 32], BF, kind="ExternalInput")
    msk_d = nc.dram_tensor("msk", [128, 2 * 256], BF, kind="ExternalInput")
    y_d = nc.dram_tensor("y", [512, D], BF, kind="ExternalOutput")
    if DEBUG:
        dbg_qtkt = nc.dram_tensor("dbg_qtkt", [128, 4 * S], BF, kind="ExternalOutput")
        dbg_vo = nc.dram_tensor("dbg_vo", [128, NSC * HC * 65], BF, kind="ExternalOutput")
        dbg_aot = nc.dram_tensor("dbg_aot", [128, 2 * S], BF, kind="ExternalOutput")
        dbg_pt = nc.dram_tensor("dbg_pt", [128, 2 * NSC * 256], BF, kind="ExternalOutput")
        dbg_st = nc.dram_tensor("dbg_st", [128, 1024], mybir.dt.float32, kind="ExternalOutput")
        dbg_ao = nc.dram_tensor("dbg_ao", [65, 1024], mybir.dt.float32, kind="ExternalOutput")
        dbg_rep = nc.dram_tensor("dbg_rep", [128, 1024], mybir.dt.float32, kind="ExternalOutput")
    cc_in = nc.dram_tensor("cc_in", [S, D], BF)
    cc_out = nc.dram_tensor("cc_out", [512, D], BF)
    groups = [[0, 1, 2, 3], [4, 5, 6, 7]]

    with tile.TileContext(nc) as tc:
        with (
            tc.tile_pool(name="const", bufs=1) as cp,
            tc.tile_pool(name="persist", bufs=1) as bp,
            tc.tile_pool(name="rope", bufs=2) as rp,
            tc.tile_pool(name="pt", bufs=2) as ptp,
            tc.tile_pool(name="rec", bufs=2) as rcp,
            tc.tile_pool(name="stage", bufs=3) as stg,
            tc.tile_pool(name="big", bufs=2, space="PSUM") as bigp,
            tc.tile_pool(name="trp", bufs=2, space="PSUM") as trp,
            tc.tile_pool(name="aop", bufs=2, space="PSUM") as aop,
        ):
            # ---- constants / weights (bf16)
            wqk_t = cp.tile([128, 8, 2 * E], BF, tag="wqk")
            wv_t = cp.tile([128, 8, E], BF, tag="wv")
            wo_t = cp.tile([128, 2, D], BF, tag="wo")
            for c2 in range(4):
                nc.gpsimd.dma_start(
                    wqk_t[:, 2 * c2:2 * c2 + 2, :],
                    wqk_d.rearrange("(c p) e -> p c e", p=128)[:, 2 * c2:2 * c2 + 2, :])
            nc.gpsimd.dma_start(wv_t[:], wv_d.rearrange("(c p) e -> p c e", p=128))
            nc.gpsimd.dma_start(wo_t[:], wo_d.rearrange("(c p) e -> p c e", p=128))
            # small tables first: chunk 0's RoPE needs them immediately
            cost = cp.tile([128, NSC, 32], BF, tag="cost")
            sint = cp.tile([128, NSC, 32], BF, tag="sint")
            nsint = cp.tile([128, NSC, 32], BF, tag="nsint")
            masks = cp.tile([128, 2, 256], BF, tag="masks")
            nc.sync.dma_start(cost[:].rearrange("p c f -> p (c f)"), cos_d[:])
            nc.sync.dma_start(sint[:].rearrange("p c f -> p (c f)"), sin_d[:])
            nc.sync.dma_start(nsint[:].rearrange("p c f -> p (c f)"), nsin_d[:])
            nc.sync.dma_start(masks[:].rearrange("p a b -> p (a b)"), msk_d[:])
            ident = cp.tile([128, 128], BF, tag="ident")
            make_identity(nc, ident[:])
            ones_sel = cp.tile([1, 128], BF, tag="ones_sel")
            nc.vector.memset(ones_sel[:], 1.0)

            # x^T resident in SBUF: [d-part, d-chunk, s]
            xt = bp.tile([128, 8, S], BF, tag="xt")
            for sc2 in range(8):
                nc.sync.dma_start(
                    xt[:, :, sc2 * 256:(sc2 + 1) * 256],
                    xt_d.rearrange("(c p) s -> p c s",
                                   p=128)[:, :, sc2 * 256:(sc2 + 1) * 256])

            # persistent activations
            # QTKT: [p, 4, S]: 0=Q feats 0:128 (heads 0,1), 1=Q feats 128:256,
            #                  2=K feats 0:128, 3=K feats 128:256
            QTKT = bp.tile([128, 4, S], BF, tag="qtkt")
            VO = bp.tile([128, NSC, HC, 65], BF, tag="vo")
            AOT = [bp.tile([128, S], BF, tag=f"aot{i}", name=f"aot{i}")
                   for i in range(2)]
            nc.vector.memset(VO[:, :, :, 64:65], 1.0)

            def phase_a(sc):
                qkp = bigp.tile([128, 1024], F32, tag="big", name="qkp")
                for dc in range(8):
                    nc.tensor.matmul(qkp[:, 0:512], xt[:, dc, sc * 128:(sc + 1) * 128],
                                     wqk_t[:, dc, :], start=(dc == 0), stop=(dc == 7))
                    nc.tensor.matmul(qkp[:, 512:768], xt[:, dc, sc * 128:(sc + 1) * 128],
                                     wv_t[:, dc, :], start=(dc == 0), stop=(dc == 7))
                # RoPE on q|k together: [128, 512]
                cosb = cost[:, sc, :].rearrange("p (a f) -> p a f", a=1) \
                    .to_broadcast([128, 16, 32])
                sinb = sint[:, sc, :].rearrange("p (a f) -> p a f", a=1) \
                    .to_broadcast([128, 8, 32])
                nsinb = nsint[:, sc, :].rearrange("p (a f) -> p a f", a=1) \
                    .to_broadcast([128, 8, 32])
                t2 = rp.tile([128, 512], BF, tag="t2")
                u2 = rp.tile([128, 512], BF, tag="u2")
                src = qkp[:, 0:512]
                with nc.allow_low_precision(reason="bf16 rope"):
                    nc.vector.tensor_tensor(
                        out=t2[:].rearrange("p (a f) -> p a f", f=32),
                        in0=src.rearrange("p (a f) -> p a f", f=32),
                        in1=cosb, op=ALU.mult)
                    s4 = src.rearrange("p (h two f) -> p h two f", two=2, f=32)
                    u4 = u2[:].rearrange("p (h two f) -> p h two f", two=2, f=32)
                    nc.vector.tensor_tensor(out=u4[:, :, 0, :], in0=s4[:, :, 1, :],
                                            in1=nsinb, op=ALU.mult)
                    nc.vector.tensor_tensor(out=u4[:, :, 1, :], in0=s4[:, :, 0, :],
                                            in1=sinb, op=ALU.mult)
                    # t2 += u2 (bf16 psum transpose cannot accumulate)
                    nc.vector.tensor_tensor(out=t2[:], in0=t2[:], in1=u2[:],
                                            op=ALU.add)
                tr = trp.tile([128, 512], BF, tag="tr", name="tr_a")
                for q in range(4):
                    nc.tensor.transpose(tr[:, q * 128:(q + 1) * 128],
                                        t2[:, q * 128:(q + 1) * 128], ident[:])
                with nc.allow_low_precision(reason="bf16 store"):
                    nc.vector.tensor_copy(
                        QTKT[:, :, sc * 128:(sc + 1) * 128],
                        tr[:].rearrange("p (a f) -> p a f", f=128))
                    # V into 65-wide head groups
                    nc.vector.tensor_copy(
                        VO[:, sc, :, 0:64],
                        qkp[:, 512:768].rearrange("p (h f) -> p h f", f=64))

            def phase_b(m):
                nkc = 2 * m + 2
                dctx = {}
                for p in range(2):  # head pair: heads (2p, 2p+1) locally
                    pt = ptp.tile([128, 2, NSC, 256], BF, tag="pt")
                    ao = aop.tile([65, 512], F32, tag="ao")
                    if DEBUG and m == 0 and p == 0:
                        DBG["pt"] = pt
                    for kc2 in range(0, nkc, 2):
                        st = bigp.tile([128, 1024], F32, tag="big", name="st")
                        for j in range(2):
                            kc = kc2 + j
                            # row-tiled pair: head 2p on rows 0:64 -> bank0,
                            # head 2p+1 on rows 64:128 -> bank1 (concurrent)
                            nc.tensor.matmul(
                                st[:, j * 256:(j + 1) * 256],
                                QTKT[0:64, 2 + p, kc * 128:(kc + 1) * 128],
                                QTKT[0:64, p, m * 256:(m + 1) * 256],
                                start=True, stop=True)
                            nc.tensor.matmul(
                                st[:, 512 + j * 256:512 + (j + 1) * 256],
                                QTKT[64:128, 2 + p, kc * 128:(kc + 1) * 128],
                                QTKT[64:128, p, m * 256:(m + 1) * 256],
                                start=True, stop=True)
                        with nc.allow_low_precision(reason="bf16 probs"):
                            nc.scalar.activation(
                                pt[:, :, kc2:kc2 + 2, :],
                                st[:].rearrange("p (h j q) -> p h j q", h=2, q=256),
                                AF.Exp, scale=1.0 / math.sqrt(DK))
                    # causal mask on the diagonal chunk-pair (multiplicative)
                    with nc.allow_low_precision(reason="bf16 mask"):
                        nc.vector.tensor_tensor(
                            out=pt[:, :, 2 * m:2 * m + 2, :],
                            in0=pt[:, :, 2 * m:2 * m + 2, :],
                            in1=masks[:].rearrange("p a b -> p () a b")
                                .to_broadcast([128, 2, 2, 256]),
                            op=ALU.mult)
                    # one accumulation group at a time per bank (interleaved
                    # groups in a shared bank corrupt the first group)
                    for h01 in range(2):
                        for kc in range(nkc):
                            nc.tensor.matmul(ao[0:65, h01 * 256:h01 * 256 + 256],
                                             VO[:, kc, 2 * p + h01, :],
                                             pt[:, h01, kc, :],
                                             start=(kc == 0), stop=(kc == nkc - 1))
                    # denominator row -> sbuf bf16 (custom-DVE ops misread psum
                    # base partition 64, and the rep matmul needs sbuf moving)
                    d2 = rcp.tile([1, 512], BF, tag="d2", name=f"d2_{p}")
                    with nc.allow_low_precision(reason="bf16 denom"):
                        nc.vector.tensor_copy(d2[:], ao[64:65, :])
                    dctx[p] = (ao, d2)
                # normalization: broadcast denom via K=1 matmul, then one
                # reciprocal + multiply; rep MMs issued after both pairs' PV
                # so the d2 copies are long done (no PE stall)
                reps = []
                for p in range(2):
                    ao, d2 = dctx[p]
                    rep = trp.tile([128, 512], F32, tag="tr", name="rep")
                    nc.tensor.matmul(rep[:], ones_sel[:], d2[:],
                                     start=True, stop=True)
                    reps.append(rep)
                for p in range(2):
                    ao, d2 = dctx[p]
                    rep_sb = rcp.tile([128, 512], F32, tag="rep_sb")
                    with nc.allow_low_precision(reason="bf16 attention out"):
                        nc.vector.reciprocal_approx_fast(out=rep_sb[:],
                                                         in_=reps[p][:])
                        nc.vector.tensor_tensor(
                            out=AOT[p][0:64, m * 256:(m + 1) * 256],
                            in0=ao[0:64, 0:256], in1=rep_sb[0:64, 0:256],
                            op=ALU.mult)
                        nc.vector.tensor_tensor(
                            out=AOT[p][64:128, m * 256:(m + 1) * 256],
                            in0=ao[0:64, 256:512], in1=rep_sb[0:64, 256:512],
                            op=ALU.mult)

            def oproj_rs(m):
                # O-projection for the two s-chunks of this q-tile
                for i, scl in enumerate((2 * m, 2 * m + 1)):
                    op = bigp.tile([128, 1024], F32, tag="big", name="op")
                    for nb in range(2):
                        for cc in range(2):
                            nc.tensor.matmul(
                                op[:, nb * 512:(nb + 1) * 512],
                                AOT[cc][:, scl * 128:(scl + 1) * 128],
                                wo_t[:, cc, nb * 512:(nb + 1) * 512],
                                start=(cc == 0), stop=(cc == 1))
                    outs = stg.tile([128, 1024], BF, tag="stage")
                    with nc.allow_low_precision(reason="bf16 output"):
                        if i == 0:
                            nc.scalar.copy(outs[:], op[:])
                        else:
                            nc.vector.tensor_copy(outs[:], op[:])
                    nc.sync.dma_start(cc_in[scl * 128:(scl + 1) * 128, :], outs[:])
                    if m == NQT - 1:
                        # final qtile: reduce-scatter per s-chunk to cut the
                        # serial tail roughly in half
                        nc.gpsimd.collective_compute(
                            "ReduceScatter", ALU.add, replica_groups=groups,
                            ins=[cc_in[128 * scl:128 * (scl + 1), :]],
                            outs=[cc_out[32 * scl:32 * (scl + 1), :]])
                        nc.sync.dma_start(y_d[32 * scl:32 * (scl + 1), :],
                                          cc_out[32 * scl:32 * (scl + 1), :])
                if m < NQT - 1:
                    # per-qtile reduce-scatter (256 rows -> 64 rows)
                    nc.gpsimd.collective_compute(
                        "ReduceScatter", ALU.add, replica_groups=groups,
                        ins=[cc_in[256 * m:256 * (m + 1), :]],
                        outs=[cc_out[64 * m:64 * (m + 1), :]])
                    nc.sync.dma_start(y_d[64 * m:64 * (m + 1), :],
                                      cc_out[64 * m:64 * (m + 1), :])

            DBG = {}
            for m in range(NQT):
                phase_a(2 * m)
                phase_a(2 * m + 1)
                if m >= 1:
                    oproj_rs(m - 1)
                phase_b(m)
            oproj_rs(NQT - 1)
            if DEBUG:
                nc.sync.dma_start(dbg_qtkt[:], QTKT[:].rearrange("p a s -> p (a s)"))
                nc.sync.dma_start(dbg_vo[:], VO[:].rearrange("p a b c -> p (a b c)"))
                for i in range(2):
                    nc.sync.dma_start(dbg_aot[:, i * S:(i + 1) * S], AOT[i][:])
                if "pt" in DBG:
                    nc.sync.dma_start(dbg_pt[:], DBG["pt"][:].rearrange("p a b c -> p (a b c)"))
                if "st" in DBG:
                    nc.sync.dma_start(dbg_st[:], DBG["st"][:])

    nc.compile()
    return nc


def _get_compiled():
    global _compiled
    if _compiled is None:
        _compiled = _build()
    return _compiled


def kernel(x, Wq, Wk, Wv, Wo, token_positions):
    from concourse.bass_utils import run_bass_kernel_spmd
    import ml_dtypes

    nc = _get_compiled()
    BF = ml_dtypes.bfloat16

    x = np.asarray(x, np.float32)
    Wq = np.asarray(Wq, np.float32)
    Wk = np.asarray(Wk, np.float32)
    Wv = np.asarray(Wv, np.float32)
    Wo = np.asarray(Wo, np.float32)
    pos = np.asarray(token_positions).astype(np.float32)  # [S]

    # rotate-half permutation within each head: [evens, odds]
    perm = np.concatenate([np.arange(0, DK, 2), np.arange(1, DK, 2)])
    inv_freq = (ROPE_THETA ** (-np.arange(0, DK, 2, dtype=np.float64) / DK))
    ang = pos[:, None].astype(np.float64) * inv_freq[None, :]  # [S, 32]
    cosT = np.cos(ang).astype(np.float32)  # [S, 32]
    sinT = np.sin(ang).astype(np.float32)
    # [S, 32] -> [128, NSC*32]: partition = s within chunk, chunk-major free
    def tab(t):
        return np.ascontiguousarray(
            t.reshape(NSC, 128, 32).transpose(1, 0, 2).reshape(128, NSC * 32)
        ).astype(BF)
    cosb, sinb, nsinb = tab(cosT), tab(sinT), tab(-sinT)

    kl = np.arange(128)[:, None]
    ql = np.arange(256)[None, :]
    m0 = (kl <= ql).astype(np.float32)
    m1 = (kl + 128 <= ql).astype(np.float32)
    msk = np.ascontiguousarray(np.concatenate([m0, m1], axis=1)).astype(BF)

    in_maps = []
    for c in range(8):
        b, g = c // 4, c % 4
        heads = range(HC * g, HC * (g + 1))
        rowsel = np.concatenate([h * DK + perm for h in heads])
        block = slice(E * g, E * (g + 1))
        wq = Wq[rowsel, :].T  # [1024, 256] perm'd output features
        wk = Wk[rowsel, :].T
        wqk = np.ascontiguousarray(np.concatenate([wq, wk], axis=1)).astype(BF)
        in_maps.append({
            "xt": np.ascontiguousarray(x[b].T).astype(BF),
            "wqk": wqk,
            "wv": np.ascontiguousarray(Wv[block, :].T).astype(BF),
            "wo": np.ascontiguousarray(Wo[:, block].T).astype(BF),
            "cost": cosb, "sint": sinb, "nsint": nsinb, "msk": msk,
        })

    res = run_bass_kernel_spmd(nc, in_maps, core_ids=list(range(8)))

    out = np.empty((B, S, D), np.float32)
    for b in range(B):
        for r in range(4):
            shard = np.asarray(res.results[4 * b + r]["y"]).astype(np.float32)
            for m in range(7):
                out[b, 256 * m + 64 * r: 256 * m + 64 * (r + 1), :] = \
                    shard[64 * m:64 * (m + 1), :]
            for scl in (14, 15):  # last qtile was reduce-scattered per s-chunk
                out[b, 128 * scl + 32 * r: 128 * scl + 32 * (r + 1), :] = \
                    shard[32 * scl:32 * (scl + 1), :]
    return out
